# revision 2
# baseline (speedup 1.0000x reference)
"""Trainium2 Bass kernel for nn_MCFL_49254684950998 — v2 (fp8 DoubleRow GEMMs).

Strategy: pure data parallel over 8 NeuronCores (batch 16384 -> 2048/core).
Feature-major layout. All five GEMMs (qkv, sa_proj, cq, ckv, ca_proj) run in
fp8-e4m3 with perf_mode=DoubleRow (2 fp8 contraction rows per PE pass ->
measured 2.04x over bf16 at FD=512). Weights host-scaled by 64 into e4m3
normal range; 1/64 folded into the PSUM->SBUF copies / residual adds.
Attention numerics (products, softmax, LayerNorm) stay bf16/f32 exactly as
the v1 kernel. Weights are SBUF-resident (loaded once, fp8 = 8MB).
"""

import sys

sys.path.insert(0, "/opt/trn_rl_repo")

import numpy as np
import ml_dtypes

import concourse.bass as bass
import concourse.bacc as bacc
import concourse.tile as tile
import concourse.mybir as mybir
from concourse import bass_utils

F32 = mybir.dt.float32
BF16 = mybir.dt.bfloat16
FP8 = mybir.dt.float8e4
DR = mybir.MatmulPerfMode.DoubleRow
AF = mybir.ActivationFunctionType
OP = mybir.AluOpType

B, D, H, HD = 16384, 1024, 16, 64
NCORES = 8
BLOC = B // NCORES          # 2048 batch rows per core
BF = 512                    # batch tile (free dim) per block
NBLK_HW = BLOC // BF        # 4 blocks per core
NCH = D // 128              # 8 feature chunks
NKK = D // 256              # 4 fp8 DoubleRow K-superchunks
SCALE = HD ** -0.5
EPS = 1e-5
WS = 64.0                   # fp8 weight scale
IWS = float(1.0 / WS)
DEVS = 8.0                  # dev-path extra scale (fp8 subnormal avoidance)
IWS8 = float(1.0 / (WS * DEVS))


def build(tc, outs, ins, nblk, flags):
    from contextlib import ExitStack
    stack = ExitStack()
    nc = tc.nc
    out_t = outs["out_t"]
    xt = [ins["xt_t"], ins["xt_i"], ins["xt_a"]]
    x8 = [ins["x8_t"], ins["x8_i"], ins["x8_a"]]
    ln1_aff, ln2_aff, sab_nz, cab_nz = (
        flags["ln1_aff"], flags["ln2_aff"], flags["sab_nz"], flags["cab_nz"])

    # ---- const tiles + resident weights (loaded once) ----
    cpool = stack.enter_context(tc.tile_pool(name="consts", bufs=1))
    sel_sb = cpool.tile([128, NCH * 16], BF16, tag="sel")       # [128, c, 16]
    nc.sync.dma_start(sel_sb[:], ins["sel"][:])
    selb_sb = cpool.tile([96, NCH * 128], BF16, tag="selb")     # bases 0/32/64
    nc.sync.dma_start(selb_sb[:], ins["selb"][:])
    ones_sb = cpool.tile([128, 1], BF16, tag="ones")            # 1/1024
    nc.sync.dma_start(ones_sb[:], ins["ones_col"][:])
    onesrow_sb = cpool.tile([65, 128], BF16, tag="onesrow")     # 1.0 @ rows 0/32/64
    nc.sync.dma_start(onesrow_sb[:], ins["ones_row"][:])
    ident_sb = cpool.tile([128, 128], BF16, tag="ident")        # I_128
    nc.sync.dma_start(ident_sb[:], ins["ident"][:])
    cols = {}
    for nm in ("sab", "l1g", "l1b", "cab", "l2g", "l2b"):
        cols[nm] = cpool.tile([128, NCH], F32, tag=nm, name=f"col_{nm}")
        nc.sync.dma_start(cols[nm][:], ins[nm][:])

    # fp8 DR weight slabs, resident, flat tiles + rearranged views.
    # qkv host layout [p, kk, o, i, m] (kk-major so the 4 tiles load in order);
    # o = c*3 + part (part 0=q,1=k,2=v), 24 o-chunks.
    wq_t = []
    for kk in range(NKK):
        t = cpool.tile([128, 24 * 256], FP8, tag=f"wqkv_{kk}")
        nc.gpsimd.dma_start(t[:], ins["wqkv8"][:, kk * 24 * 256:(kk + 1) * 24 * 256])
        wq_t.append(t[:].rearrange("p (o i m) -> p o i m", o=24, i=2))

    def res_w(name, no):
        t = cpool.tile([128, no * NKK * 256], FP8, tag=name)
        nc.gpsimd.dma_start(t[:], ins[name][:])
        return t[:].rearrange("p (o kk i m) -> p o kk i m", o=no, kk=NKK, i=2)

    wsa_sb = res_w("wsa8", NCH)
    wcq_sb = res_w("wq8", NCH)
    wkv_sb = res_w("wkv8", 16)
    wca_sb = res_w("wca8", NCH)

    def sel_c(c):
        return sel_sb[:, c * 16:(c + 1) * 16]

    def selb_c(c, base=0):
        return selb_sb[base:base + 16, c * 128:(c + 1) * 128]

    def pool(*a, **k):
        return stack.enter_context(tc.tile_pool(*a, **k))

    # pair tiles [128, 2, BF] fp8; tags cycle x8 (head) -> aop (phase4) -> t8 (LN1)
    p8_pool = pool(name="p8", bufs=2)
    x_pool = pool(name="xs", bufs=1)          # 24 tags [128,BF] bf16
    qk_pool = pool(name="qk", bufs=10)         # [128, BF] bf16
    v_pool = pool(name="vs", bufs=1)          # 24 tags [128,BF] bf16 (reused for cv)
    pr_pool = pool(name="prod", bufs=2)       # [128, BF] bf16
    tt_pool = pool(name="tt", bufs=4)         # [128, BF] bf16
    w2_pool = pool(name="w2sl", bufs=1)       # [128, 8*128] bf16 W2 slabs
    xts_pool = pool(name="xts", bufs=1)       # 8 tags: xsum -> msb -> tsum -> msc
    caop_pool = pool(name="caop", bufs=1)     # 4 tags [128,2,BF] fp8
    aop_pool = pool(name="aop", bufs=1)       # 12 tags [128,2,BF] fp8
    msb_pool = pool(name="msb", bufs=1)       # 8 tags [128,BF] bf16
    sm_pool = pool(name="sm", bufs=1)         # small f32/bf16 softmax+LN tiles
    pp_pool = pool(name="pp", bufs=2)         # softmax P tiles
    rbc_pool = pool(name="rbc", bufs=1)       # [128,BF] bf16 rstd broadcast
    ps_big = pool(name="psbig", bufs=5, space="PSUM")
    ps_S = pool(name="psS", bufs=3, space="PSUM")

    def ln_stats(stA, stB, t, c, y):
        """One chunk's LN stat matmuls (col-group packed at partition 32t)."""
        b0 = 32 * t
        sq = tt_pool.tile([128, BF], BF16, tag="tt", name="sq")
        nc.vector.tensor_tensor(sq[:], y, y, op=OP.mult)
        st, sp = (c == 0), (c == NCH - 1)
        nc.tensor.matmul(stA[b0:b0 + 1, :], ones_sb[:], y,
                         start=st, stop=sp, tile_position=(0, b0),
                         skip_group_check=True)
        nc.tensor.matmul(stB[b0:b0 + 1, :], ones_sb[:], sq[:],
                         start=st, stop=sp, tile_position=(0, b0),
                         skip_group_check=True)

    def ln_finish(stA, stB, ylists, gcol, bcol, dstlists, affine):
        """Smalls + rank-1 broadcasts + apply, after all ln_stats chunks."""
        ntok = len(ylists)
        R = 32 * (ntok - 1) + 1
        mu_s = sm_pool.tile([65, BF], BF16, tag="mu_s")
        nc.vector.tensor_copy(mu_s[0:R, :], stA[0:R, :])
        mu2 = sm_pool.tile([65, BF], BF16, tag="lnvr", name="mu2")
        nc.vector.tensor_tensor(mu2[0:R, :], mu_s[0:R, :], mu_s[0:R, :], op=OP.mult)
        var = sm_pool.tile([65, BF], BF16, tag="var")
        nc.vector.scalar_tensor_tensor(var[0:R, :], stB[0:R, :], EPS, mu2[0:R, :],
                                       op0=OP.add, op1=OP.subtract)
        lnv = sm_pool.tile([65, BF], BF16, tag="lnvr")
        nc.scalar.activation(lnv[0:R, :], var[0:R, :], AF.Ln)
        rstd = sm_pool.tile([65, BF], BF16, tag="rstd")
        nc.scalar.activation(rstd[0:R, :], lnv[0:R, :], AF.Exp, scale=-0.5)
        mup = sm_pool.tile([65, BF], BF16, tag="mup")
        nc.vector.tensor_tensor(mup[0:R, :], mu_s[0:R, :], rstd[0:R, :], op=OP.mult)
        for t in range(ntok):
            b0 = 32 * t
            rb_ps = ps_S.tile([128, BF], F32, tag="S", name="rb_ps")
            nc.tensor.matmul(rb_ps[:], onesrow_sb[b0:b0 + 1, :],
                             rstd[b0:b0 + 1, :], start=True, stop=True)
            rb = rbc_pool.tile([128, BF], BF16, tag="rbc")
            nc.scalar.copy(rb[:], rb_ps[:])
            mu_ps = ps_S.tile([128, BF], F32, tag="S", name="mu_ps")
            nc.tensor.matmul(mu_ps[:], onesrow_sb[b0:b0 + 1, :],
                             mup[b0:b0 + 1, :], start=True, stop=True)
            for c in range(NCH):
                t1 = tt_pool.tile([128, BF], BF16, tag="tt")
                nc.vector.tensor_tensor(t1[:], ylists[t][c], rb[:], op=OP.mult)
                if affine:
                    t2 = tt_pool.tile([128, BF], BF16, tag="tt")
                    nc.vector.tensor_tensor(t2[:], t1[:], mu_ps[:], op=OP.subtract)
                    nc.vector.tensor_scalar(dstlists[t][c], t2[:], gcol[:, c:c + 1],
                                            bcol[:, c:c + 1], op0=OP.mult, op1=OP.add)
                else:
                    nc.vector.tensor_tensor(dstlists[t][c], t1[:], mu_ps[:],
                                            op=OP.subtract)

    def head(blk):
        """x loads, fp8-DR qkv GEMM + scores, softmax -> P tiles."""
        bs = blk * BF
        qs = (nc.sync, nc.scalar)
        # bf16 x tiles (residual / LN path) — first: their WAR deps clear
        # earliest and the gpsimd xsum chain hangs off them
        xs = {}
        for m in range(3):
            for c in range(NCH):
                t = x_pool.tile([128, BF], BF16, tag=f"x_{m}_{c}")
                qs[(m * NCH + c) % 2].dma_start(
                    t[:], xt[m][c * 128:(c + 1) * 128, bs:bs + BF])
                xs[(m, c)] = t
        # fp8 pair tiles for the qkv moving operand
        x8t = {}
        for m in range(3):
            for kk in range(NKK):
                t = p8_pool.tile([128, 2, BF], FP8, tag=f"p8_{m}_{kk}",
                                 name=f"x8_{m}_{kk}")
                for i in range(2):
                    qs[(m * NKK + kk + i) % 2].dma_start(
                        t[:, i, :],
                        x8[m][(2 * kk + i) * 128:(2 * kk + i + 1) * 128, bs:bs + BF])
                x8t[(m, kk)] = t

        # xsum = x_t + x_i + x_a, host-precomputed, DMA'd like the x tiles
        xsum = {}
        for c in range(NCH):
            t = xts_pool.tile([128, BF], BF16, tag=f"xts_{c}", name=f"xsum_{c}")
            qs[c % 2].dma_start(t[:], ins["xsum"][c * 128:(c + 1) * 128, bs:bs + BF])
            xsum[c] = t
        S_banks = [ps_S.tile([128, BF], F32, tag="S", name=f"Sbank{i}")
                   for i in range(3)]
        vs = {}

        def emit_products(c, qts, kts):
            for i in range(3):
                for j in range(3):
                    pr = pr_pool.tile([128, BF], BF16, tag="prod")
                    nc.vector.tensor_tensor(pr[:], qts[i][:], kts[j][:], op=OP.mult)
                    nc.tensor.matmul(
                        S_banks[j][32 * i:32 * i + 16, :],
                        sel_c(c), pr[:],
                        start=(c == 0), stop=(c == NCH - 1),
                        tile_position=(0, 32 * i),
                        skip_group_check=True,
                    )

        # software-pipelined: chunk c's products/sel-MMs are emitted after
        # chunk c+1's GEMM matmuls so the PE never waits on the DVE products.
        # m_sa = xsum @ W2 (bf16, exact) interleaved one o-chunk per iteration
        # from c=1 (xsum is gpsimd-ready by then).
        msb = {}

        def emit_msa(o):
            w2 = w2_pool.tile([128, NCH * 128], BF16, tag="w2sl", name=f"w2_{o}")
            nc.gpsimd.dma_start(w2[:], ins["w2"][:, o * 1024:(o + 1) * 1024])
            w2v = w2[:].rearrange("p (k m) -> p k m", k=NCH)
            psm = ps_big.tile([128, BF], F32, tag="big", name=f"psm_{o}")
            for k in range(NCH):
                nc.tensor.matmul(psm[:], w2v[:, k], xsum[k][:],
                                 start=(k == 0), stop=(k == NCH - 1))
            mt = msb_pool.tile([128, BF], BF16, tag=f"msb_{o}", name=f"msa_{o}")
            nc.scalar.copy(mt[:], psm[:])
            msb[o] = mt

        pending = None
        for c in range(NCH):
            parts = []
            for part in range(3):
                pss = [ps_big.tile([128, BF], F32, tag="big", name=f"pss{_i}") for _i in range(3)]
                for kk in range(NKK):
                    wap = wq_t[kk][:, c * 3 + part]
                    for m in range(3):
                        nc.tensor.matmul(pss[m][:], wap, x8t[(m, kk)][:],
                                         start=(kk == 0), stop=(kk == NKK - 1),
                                         perf_mode=DR)
                outs_p = []
                for m in range(3):
                    if part == 2:
                        t = v_pool.tile([128, BF], BF16, tag=f"v_{m}_{c}")
                        vs[(m, c)] = t
                    else:
                        t = qk_pool.tile([128, BF], BF16, tag="qk")
                    nc.scalar.activation(t[:], pss[m][:], AF.Copy, scale=IWS)
                    outs_p.append(t)
                parts.append(outs_p)
            if pending is not None:
                emit_products(*pending)
            pending = (c, parts[0], parts[1])
        emit_products(*pending)
        for o in range(NCH):
            emit_msa(o)

        def warm(src):
            mv = src[:].bitcast(BF16) if src.dtype == F32 else src[:]
            nc.tensor.matmul(S_banks[0][96:112, 0:64], sel_sb[0:80, 0:16],
                             mv[0:80, 0:64], start=True, stop=True,
                             tile_position=(0, 96), skip_group_check=True)

        E = []
        for j in range(3):
            e = qk_pool.tile([80, BF], BF16, tag="qk", name=f"E{j}")
            nc.scalar.activation(e[:], S_banks[j][0:80, :], AF.Exp)
            E.append(e)
        warm(E[0])
        esum = sm_pool.tile([80, BF], F32, tag="esum")
        nc.vector.tensor_tensor(esum[:], E[0][:], E[1][:], op=OP.add)
        nc.vector.tensor_tensor(esum[:], esum[:], E[2][:], op=OP.add)
        warm(esum)
        rec = sm_pool.tile([80, BF], F32, tag="rec")
        nc.vector.reciprocal_approx_fast(rec[:], esum[:])
        warm(rec)
        # P' = (P - 1/3) * DEVS  (mean-compensated, scaled into e4m3 range)
        P = []
        for j in range(3):
            p = pp_pool.tile([80, BF], BF16, tag=f"P{j}", name=f"P{j}")
            nc.vector.tensor_tensor(p[:], E[j][:], rec[:], op=OP.mult)
            nc.vector.tensor_scalar(p[:], p[:], -1.0 / 3.0, DEVS,
                                    op0=OP.add, op1=OP.mult)
            P.append(p)
        return dict(bs=bs, xs=xs, vs=vs, P=P, msb=msb)

    def tail(st):
        bs, xs, vs, P, msb = st["bs"], st["xs"], st["vs"], st["P"], st["msb"]
        # ---- phase 4: dev attnout (P' weights) -> fp8 pair tiles ----
        aop = {}
        for tok in range(3):
            b0 = 32 * tok
            for kk in range(NKK):
                aop[(tok, kk)] = aop_pool.tile([128, 2, BF], FP8,
                                               tag=f"aop_{tok}_{kk}",
                                               name=f"aop_{tok}_{kk}")
            for c in range(NCH):
                ts = []
                for j in range(3):
                    pe = ps_big.tile([128, BF], F32, tag="big")
                    nc.tensor.matmul(pe[:], selb_c(c, b0),
                                     P[j][b0:b0 + 16, :], start=True, stop=True)
                    t = tt_pool.tile([128, BF], BF16, tag="tt")
                    nc.vector.tensor_tensor(t[:], pe[:], vs[(j, c)][:], op=OP.mult)
                    ts.append(t)
                t01 = tt_pool.tile([128, BF], BF16, tag="tt")
                nc.gpsimd.tensor_tensor(t01[:], ts[0][:], ts[1][:], op=OP.add)
                nc.gpsimd.tensor_tensor(aop[(tok, kk := c // 2)][:, c % 2, :],
                                        t01[:], ts[2][:], op=OP.add)
        # sa_proj (dev, fp8 DR) + identity-MM mean inject + residual,
        # with LN1 stats interleaved right after each chunk's residual
        stA1 = ps_S.tile([128, BF], F32, tag="S", name="stA1")
        stB1 = ps_S.tile([128, BF], F32, tag="S", name="stB1")
        for o in range(NCH):
            pss = [ps_big.tile([128, BF], F32, tag="big", name=f"pss{_i}") for _i in range(3)]
            for kk in range(NKK):
                wap = wsa_sb[:, o, kk]
                for tok in range(3):
                    nc.tensor.matmul(pss[tok][:], wap, aop[(tok, kk)][:],
                                     start=(kk == 0), stop=False,
                                     perf_mode=DR)
            for tok in range(3):
                nc.tensor.matmul(pss[tok][:], ident_sb[:], msb[o][:],
                                 start=False, stop=True)
            for tok in range(3):
                nc.vector.scalar_tensor_tensor(
                    xs[(tok, o)][:], pss[tok][:], IWS8, xs[(tok, o)][:],
                    op0=OP.mult, op1=OP.add)
                if sab_nz:
                    nc.vector.tensor_scalar_add(
                        xs[(tok, o)][:], xs[(tok, o)][:], cols["sab"][:, o:o + 1])
                ln_stats(stA1, stB1, tok, o, xs[(tok, o)][:])

        # ---- phase 5: LN1 finish (stats already accumulated) ----
        ln_finish(stA1, stB1,
                  [[xs[(m, c)][:] for c in range(NCH)] for m in range(3)],
                  cols["l1g"], cols["l1b"],
                  [[xs[(m, c)][:] for c in range(NCH)] for m in range(3)], ln1_aff)
        t8 = {}
        for m in range(3):
            for kk in range(NKK):
                t8[(m, kk)] = p8_pool.tile([128, 2, BF], FP8, tag=f"p8_{m}_{kk}",
                                           name=f"t8_{m}_{kk}")
        for m in range(3):
            for c in range(NCH):
                nc.scalar.copy(t8[(m, c // 2)][:, c % 2, :], xs[(m, c)][:])
        # tsum = t_img + t_aud (gpsimd) -> m_ca = tsum @ W2c once (bf16)
        tsum = {}
        for c in range(NCH):
            t = xts_pool.tile([128, BF], BF16, tag=f"xts_{c}", name=f"tsum_{c}")
            nc.gpsimd.tensor_tensor(t[:], xs[(1, c)][:], xs[(2, c)][:], op=OP.add)
            tsum[c] = t
        msc = {}

        def emit_msc(o):
            w2 = w2_pool.tile([128, NCH * 128], BF16, tag="w2sl", name=f"w2c_{o}")
            nc.gpsimd.dma_start(w2[:], ins["w2c"][:, o * 1024:(o + 1) * 1024])
            w2v = w2[:].rearrange("p (k m) -> p k m", k=NCH)
            psm = ps_big.tile([128, BF], F32, tag="big", name=f"psmc_{o}")
            for k in range(NCH):
                nc.tensor.matmul(psm[:], w2v[:, k], tsum[k][:],
                                 start=(k == 0), stop=(k == NCH - 1))
            mt = msb_pool.tile([128, BF], BF16, tag=f"msb_{o}", name=f"msc_{o}")
            nc.scalar.copy(mt[:], psm[:])
            msc[o] = mt

        # ---- phase 6: cross attention ----
        # cq
        cqs = {}
        for o in range(NCH):
            ps = ps_big.tile([128, BF], F32, tag="big")
            for kk in range(NKK):
                nc.tensor.matmul(ps[:], wcq_sb[:, o, kk], t8[(0, kk)][:],
                                 start=(kk == 0), stop=(kk == NKK - 1),
                                 perf_mode=DR)
            cq = v_pool.tile([128, BF], BF16, tag=f"v_0_{o}", name=f"cq_{o}")
            nc.scalar.activation(cq[:], ps[:], AF.Copy, scale=IWS)
            cqs[o] = cq
        # ck + cross scores
        Sc0 = ps_S.tile([128, BF], F32, tag="S", name="Sc0")
        Sc1 = ps_S.tile([128, BF], F32, tag="S", name="Sc1")
        Scs = [Sc0, Sc1]
        def emit_cross_scores(o, cks):
            for pi in range(2):
                pr = pr_pool.tile([128, BF], BF16, tag="prod")
                nc.vector.tensor_tensor(pr[:], cqs[o][:], cks[pi][:], op=OP.mult)
                nc.tensor.matmul(
                    Scs[pi][0:16, :], sel_c(o), pr[:],
                    start=(o == 0), stop=(o == NCH - 1))

        cpend = None
        for o in range(NCH):
            ps2 = [ps_big.tile([128, BF], F32, tag="big", name=f"ps2_{_i}") for _i in range(2)]
            for kk in range(NKK):
                wap = wkv_sb[:, o, kk]
                for pi in range(2):
                    nc.tensor.matmul(ps2[pi][:], wap, t8[(pi + 1, kk)][:],
                                     start=(kk == 0), stop=(kk == NKK - 1),
                                     perf_mode=DR)
            cks = []
            for pi in range(2):
                ck = qk_pool.tile([128, BF], BF16, tag="qk", name=f"ck_{o}_{pi}")
                nc.scalar.activation(ck[:], ps2[pi][:], AF.Copy, scale=IWS)
                cks.append(ck)
            if cpend is not None:
                emit_cross_scores(*cpend)
            cpend = (o, cks)
        emit_cross_scores(*cpend)
        # cross softmax over 2 keys; Pc' = (Pc - 1/2) * DEVS
        Ec0 = sm_pool.tile([16, BF], BF16, tag="E0", name="Ec0")
        nc.scalar.activation(Ec0[:], Sc0[0:16, :], AF.Exp)
        Ec1 = sm_pool.tile([16, BF], BF16, tag="E1", name="Ec1")
        nc.scalar.activation(Ec1[:], Sc1[0:16, :], AF.Exp)
        esc = sm_pool.tile([16, BF], F32, tag="esum", name="esc")
        nc.vector.tensor_tensor(esc[:], Ec0[:], Ec1[:], op=OP.add)
        recc = sm_pool.tile([16, BF], F32, tag="rec", name="recc")
        nc.vector.reciprocal_approx_fast(recc[:], esc[:])
        Pc0 = pp_pool.tile([16, BF], BF16, tag="P0", name="Pc0")
        nc.vector.tensor_tensor(Pc0[:], Ec0[:], recc[:], op=OP.mult)
        nc.vector.tensor_scalar(Pc0[:], Pc0[:], -0.5, DEVS, op0=OP.add, op1=OP.mult)
        Pc1 = pp_pool.tile([16, BF], BF16, tag="P1", name="Pc1")
        nc.vector.tensor_tensor(Pc1[:], Ec1[:], recc[:], op=OP.mult)
        nc.vector.tensor_scalar(Pc1[:], Pc1[:], -0.5, DEVS, op0=OP.add, op1=OP.mult)
        # cv (v-part of Wkv: o index 8..15); tiles reuse v_pool tags
        cvs = {}
        for o in range(NCH):
            if o >= 1:
                emit_msc(o - 1)
            ps2 = [ps_big.tile([128, BF], F32, tag="big", name=f"ps2_{_i}") for _i in range(2)]
            for kk in range(NKK):
                wap = wkv_sb[:, 8 + o, kk]
                for pi in range(2):
                    nc.tensor.matmul(ps2[pi][:], wap, t8[(pi + 1, kk)][:],
                                     start=(kk == 0), stop=(kk == NKK - 1),
                                     perf_mode=DR)
            for pi in range(2):
                cv = v_pool.tile([128, BF], BF16, tag=f"v_{pi + 1}_{o}",
                                 name=f"cv_{pi}_{o}")
                nc.scalar.activation(cv[:], ps2[pi][:], AF.Copy, scale=IWS)
                cvs[(pi, o)] = cv
        emit_msc(NCH - 1)
        # weighted cv sum -> cross attnout -> fp8 pairs
        caop = {}
        for kk in range(NKK):
            caop[kk] = caop_pool.tile([128, 2, BF], FP8, tag=f"caop_{kk}",
                                      name=f"caop_{kk}")
        for c in range(NCH):
            pe_i = ps_big.tile([128, BF], F32, tag="big")
            nc.tensor.matmul(pe_i[:], selb_c(c), Pc0[:], start=True, stop=True)
            pe_a = ps_big.tile([128, BF], F32, tag="big")
            nc.tensor.matmul(pe_a[:], selb_c(c), Pc1[:], start=True, stop=True)
            t0 = tt_pool.tile([128, BF], BF16, tag="tt")
            nc.vector.tensor_tensor(t0[:], pe_i[:], cvs[(0, c)][:], op=OP.mult)
            t1 = tt_pool.tile([128, BF], BF16, tag="tt")
            nc.vector.tensor_tensor(t1[:], pe_a[:], cvs[(1, c)][:], op=OP.mult)
            nc.gpsimd.tensor_tensor(caop[c // 2][:, c % 2, :], t0[:], t1[:],
                                    op=OP.add)
        # ca_proj (dev) + identity-MM mean inject + residual + LN2 stats
        stA2 = ps_S.tile([128, BF], F32, tag="S", name="stA2")
        stB2 = ps_S.tile([128, BF], F32, tag="S", name="stB2")
        for o in range(NCH):
            ps = ps_big.tile([128, BF], F32, tag="big")
            for kk in range(NKK):
                nc.tensor.matmul(ps[:], wca_sb[:, o, kk], caop[kk][:],
                                 start=(kk == 0), stop=False,
                                 perf_mode=DR)
            nc.tensor.matmul(ps[:], ident_sb[:], msc[o][:],
                             start=False, stop=True)
            nc.vector.scalar_tensor_tensor(
                xs[(0, o)][:], ps[:], IWS8, xs[(0, o)][:],
                op0=OP.mult, op1=OP.add)
            if cab_nz:
                nc.vector.tensor_scalar_add(
                    xs[(0, o)][:], xs[(0, o)][:], cols["cab"][:, o:o + 1])
            ln_stats(stA2, stB2, 0, o, xs[(0, o)][:])

        # ---- phase 7: LN2 on text token, f32 out, store ----
        outs_t = [tt_pool.tile([128, BF], BF16, tag="tt", name=f"out_{c}")
                  for c in range(NCH)]
        ln_finish(stA2, stB2, [[xs[(0, c)][:] for c in range(NCH)]],
                  cols["l2g"], cols["l2b"], [[o[:] for o in outs_t]], ln2_aff)
        for c in range(NCH):
            nc.sync.dma_start(out_t[c * 128:(c + 1) * 128, bs:bs + BF], outs_t[c][:])

    for blk in range(nblk):
        tail(head(blk))

    stack.close()


# ------------------------------------------------------------------ host side

def _dr_pack(W, no):
    """W [1024, no*128] f32 -> [128, no*NKK*256] fp8 slabs [p, o, kk, i, m]."""
    f8 = ml_dtypes.float8_e4m3
    w = np.asarray(W, np.float32) * WS
    w = w.reshape(NKK, 2, 128, no, 128)        # [kk, i, p, o, m]
    w = w.transpose(2, 3, 0, 1, 4)             # [p, o, kk, i, m]
    w = np.clip(w.reshape(128, no * NKK * 256), -240, 240)
    return np.ascontiguousarray(w).astype(f8)


def _prep_shared(Wqkv, sa_proj_w, sa_proj_b, ln1_g, ln1_b, Wq, Wkv, ca_proj_w,
                 ca_proj_b, ln2_g, ln2_b):
    f = np.float32
    bf = ml_dtypes.bfloat16

    # qkv with out-chunks ordered [c, (q_c | k_c | v_c)], SCALE folded into q
    Wq3 = np.asarray(Wqkv, f).reshape(1024, 3, NCH, 128).copy()  # [k, part, c, m]
    Wq3[:, 0] *= SCALE
    qkv_re = Wq3.transpose(0, 2, 1, 3).reshape(1024, 24 * 128)   # [k, (c part m)]

    sel = np.zeros((128, NCH, 16), f)
    for r in range(128):
        for c in range(NCH):
            sel[r, c, 2 * c + r // 64] = 1.0
    selb1 = np.zeros((16, NCH, 128), f)
    for h in range(16):
        for c in range(NCH):
            for m in range(128):
                if h == 2 * c + m // 64:
                    selb1[h, c, m] = 1.0
    selb = np.zeros((96, NCH, 128), f)
    for b0 in (0, 32, 64):
        selb[b0:b0 + 16] = selb1
    onesrow3 = np.zeros((65, 128), f)
    for b0 in (0, 32, 64):
        onesrow3[b0] = 1.0
    col = lambda v: np.ascontiguousarray(np.asarray(v, f).reshape(NCH, 128).T)
    # qkv: kk-major layout [p, kk, o, i, m]
    f8 = ml_dtypes.float8_e4m3
    wq8 = (qkv_re * WS).reshape(NKK, 2, 128, 24 * 128)   # [kk, i, p, (o m)]
    wq8 = wq8.transpose(2, 0, 1, 3).reshape(128, NKK, 2, 24, 128)  # [p,kk,i,o,m]
    wq8 = wq8.transpose(0, 1, 3, 2, 4)                   # [p, kk, o, i, m]
    wq8 = np.clip(wq8.reshape(128, 24 * NKK * 256), -240, 240)
    def w2pack(W2):
        # [1024, 1024] -> [128, (o k m)] bf16, slab per o
        w = W2.reshape(NCH, 128, NCH, 128)            # [k, p, o, m]
        w = w.transpose(1, 2, 0, 3)                   # [p, o, k, m]
        return np.ascontiguousarray(w.reshape(128, NCH * 1024)).astype(bf)

    W2 = (np.asarray(Wqkv, f)[:, 2 * D:] @ np.asarray(sa_proj_w, f)) * (WS * DEVS / 3.0)
    W2c = (np.asarray(Wkv, f)[:, D:] @ np.asarray(ca_proj_w, f)) * (WS * DEVS / 2.0)
    return {
        "wqkv8": np.ascontiguousarray(wq8).astype(f8),
        "w2": w2pack(W2),
        "w2c": w2pack(W2c),
        "wsa8": _dr_pack(np.asarray(sa_proj_w, f), NCH),
        "wq8": _dr_pack(np.asarray(Wq, f) * SCALE, NCH),
        "wkv8": _dr_pack(np.asarray(Wkv, f), 16),
        "wca8": _dr_pack(np.asarray(ca_proj_w, f), NCH),
        "sel": sel.reshape(128, NCH * 16).astype(bf),
        "selb": selb.reshape(96, NCH * 128).astype(bf),
        "ones_col": np.full((128, 1), 1.0 / D, f).astype(bf),
        "ident": np.eye(128, dtype=f).astype(bf),
        "ones_row": onesrow3.astype(bf),
        "sab": col(sa_proj_b), "l1g": col(ln1_g), "l1b": col(ln1_b),
        "cab": col(ca_proj_b), "l2g": col(ln2_g), "l2b": col(ln2_b),
    }


def _pin_act_tables(arch):
    """Resolve Exp and Ln to the one act-table set holding both, so the
    kernel needs a single table load instead of swapping per ln_group."""
    from concourse import hw_specs
    tabs = hw_specs.get_activation_tables(arch)
    if "natural_log_exp_and_others" not in tabs:
        return
    for name, s in tabs.items():
        if name != "natural_log_exp_and_others":
            s.discard(AF.Exp)
            s.discard(AF.Ln)


_CACHE = {}


def _get_program(nblk, flags):
    key = (nblk, tuple(sorted(flags.items())))
    if key in _CACHE:
        return _CACHE[key]
    nc = bacc.Bacc("TRN2", target_bir_lowering=False, debug=False,
                   enable_asserts=False, num_devices=NCORES)
    _pin_act_tables(nc.m.arch)
    ins = {}
    bl = nblk * BF
    for nm in ("xt_t", "xt_i", "xt_a", "xsum"):
        ins[nm] = nc.dram_tensor(nm, [D, bl], BF16, kind="ExternalInput").ap()
    for nm in ("x8_t", "x8_i", "x8_a"):
        ins[nm] = nc.dram_tensor(nm, [D, bl], FP8, kind="ExternalInput").ap()
    ins["wqkv8"] = nc.dram_tensor("wqkv8", [128, 24 * NKK * 256], FP8,
                                  kind="ExternalInput").ap()
    for nm, no in (("wsa8", NCH), ("wq8", NCH), ("wkv8", 16), ("wca8", NCH)):
        ins[nm] = nc.dram_tensor(nm, [128, no * NKK * 256], FP8,
                                 kind="ExternalInput").ap()
    for nm in ("w2", "w2c"):
        ins[nm] = nc.dram_tensor(nm, [128, NCH * 1024], BF16,
                                 kind="ExternalInput").ap()
    ins["sel"] = nc.dram_tensor("sel", [128, NCH * 16], BF16, kind="ExternalInput").ap()
    ins["selb"] = nc.dram_tensor("selb", [96, NCH * 128], BF16, kind="ExternalInput").ap()
    ins["ones_col"] = nc.dram_tensor("ones_col", [128, 1], BF16, kind="ExternalInput").ap()
    ins["ones_row"] = nc.dram_tensor("ones_row", [65, 128], BF16, kind="ExternalInput").ap()
    ins["ident"] = nc.dram_tensor("ident", [128, 128], BF16, kind="ExternalInput").ap()
    for nm in ("sab", "l1g", "l1b", "cab", "l2g", "l2b"):
        ins[nm] = nc.dram_tensor(nm, [128, NCH], F32, kind="ExternalInput").ap()
    outs = {"out_t": nc.dram_tensor("out_t", [D, bl], BF16, kind="ExternalOutput").ap()}

    with tile.TileContext(nc) as tc:
        build(tc, outs, ins, nblk, flags)
    nc.compile()
    _CACHE[key] = nc
    return nc


def kernel(c_text, c_image, c_audio, Wqkv, sa_proj_w, sa_proj_b, ln1_g, ln1_b,
           Wq, Wkv, ca_proj_w, ca_proj_b, ln2_g, ln2_b, _trace=False):
    bf = ml_dtypes.bfloat16
    f8 = ml_dtypes.float8_e4m3
    shared = _prep_shared(Wqkv, sa_proj_w, sa_proj_b, ln1_g, ln1_b, Wq, Wkv,
                          ca_proj_w, ca_proj_b, ln2_g, ln2_b)
    flags = {
        "ln1_aff": not (np.allclose(np.asarray(ln1_g), 1.0)
                        and np.allclose(np.asarray(ln1_b), 0.0)),
        "ln2_aff": not (np.allclose(np.asarray(ln2_g), 1.0)
                        and np.allclose(np.asarray(ln2_b), 0.0)),
        "sab_nz": not np.allclose(np.asarray(sa_proj_b), 0.0),
        "cab_nz": not np.allclose(np.asarray(ca_proj_b), 0.0),
    }
    xf = {
        "t": np.ascontiguousarray(np.asarray(c_text, np.float32).T),
        "i": np.ascontiguousarray(np.asarray(c_image, np.float32).T),
        "a": np.ascontiguousarray(np.asarray(c_audio, np.float32).T),
    }
    xT = {f"xt_{k}": v.astype(bf) for k, v in xf.items()}
    xT["xsum"] = (xf["t"] + xf["i"] + xf["a"]).astype(bf)
    x8 = {f"x8_{k}": np.clip(v, -240, 240).astype(f8) for k, v in xf.items()}
    in_maps = []
    for s in range(NCORES):
        sl = slice(s * BLOC, (s + 1) * BLOC)
        m = dict(shared)
        for k in xT:
            m[k] = np.ascontiguousarray(xT[k][:, sl])
        for k in x8:
            m[k] = np.ascontiguousarray(x8[k][:, sl])
        in_maps.append(m)
    nc = _get_program(NBLK_HW, flags)
    res = bass_utils.run_bass_kernel_spmd(nc, in_maps, core_ids=list(range(NCORES)),
                                          trace=_trace)
    out = np.concatenate([np.asarray(r["out_t"]).T for r in res.results], axis=0)
    if _trace:
        kernel.last_results = res
    return out.astype(np.float32)
